# revision 3
# baseline (speedup 1.0000x reference)
"""LSTM (T=512 final-state) + MLP head, sharded batch-parallel over 8 TRN2 NeuronCores.

Per core (B_c=32, T=512, D=768, H=128), pipelined in 32 groups of 16 timesteps:
  1. DMA x tiles [128 tok, 768] fp32 (tokens ordered parity-major for PSUM banking)
  2. cast fp32->bf16 on GPSIMD (keeps DVE/ACT free for the scan)
  3. PE-transpose x tiles into PSUM staging (bitcast-aliased into the group's
     PSUM accumulator banks before the projection overwrites them)
  4. evacuate x^T to SBUF (DVE), then projection matmuls (weights stationary,
     N=256 token streams) accumulate W_ih @ x^T + bias straight into PSUM in
     gate-major (g, step-parity, step, b) layout
  5. the recurrent scan accumulates W_hh @ h_{t-1} on top (start=False) and runs
     sigmoid/tanh + c/h updates with the batch split in two interleaved halves
     to hide the serial per-step latency
  6. tiny MLP head in fp32 at the end.

Numerics: matmul inputs bf16 (PSUM fp32 accum), gate math fp32. The tanh gate
is computed as 2*sigmoid(2x)-1 with the 2x folded into W_ih/W_hh/bias rows so
one fused sigmoid covers all four gates.
"""

import numpy as np

B, T, D, H = 256, 512, 768, 128
NCORES = 8
BC = B // NCORES          # 32 batch per core
HB = BC // 2              # 16, half-batch for scan interleaving
NG = T // 16              # 32 groups of 16 steps
F32 = "float32"

_cache = {}


def _build():
    import concourse.bass as bass
    import concourse.mybir as mybir
    import concourse.tile as tile
    from concourse import bacc
    from concourse.masks import make_identity
    from contextlib import ExitStack

    f32 = mybir.dt.float32
    bf16 = mybir.dt.bfloat16
    AF = mybir.ActivationFunctionType
    OP = mybir.AluOpType

    nc = bacc.Bacc("TRN2", debug=False, enable_asserts=False, num_devices=NCORES)

    x_d = nc.dram_tensor("x", (BC, T, D), f32, kind="ExternalInput").ap()
    wproj_d = nc.dram_tensor("wproj", (128, 4 * 6 * 128), bf16, kind="ExternalInput").ap()
    whh_d = nc.dram_tensor("whh", (128, 512), bf16, kind="ExternalInput").ap()
    biasl_d = nc.dram_tensor("biasl", (1, 512), bf16, kind="ExternalInput").ap()
    w1t_d = nc.dram_tensor("w1t", (128, 64), f32, kind="ExternalInput").ap()
    b1_d = nc.dram_tensor("b1", (64, 1), f32, kind="ExternalInput").ap()
    w2t_d = nc.dram_tensor("w2t", (64, 32), f32, kind="ExternalInput").ap()
    b2_d = nc.dram_tensor("b2", (32, 1), f32, kind="ExternalInput").ap()
    w3t_d = nc.dram_tensor("w3t", (32, 1), f32, kind="ExternalInput").ap()
    b3_d = nc.dram_tensor("b3", (1, 1), f32, kind="ExternalInput").ap()
    y_d = nc.dram_tensor("y", (1, BC), f32, kind="ExternalOutput").ap()

    # x viewed as [parity, t//2, b, d] so a group's even/odd steps DMA as
    # [4 steps, 32 b] = 128 partitions with contiguous 3KB rows.
    x_r = x_d.rearrange("b (th p) d -> p th b d", p=2)

    with ExitStack() as ctx:
        tc = ctx.enter_context(tile.TileContext(nc))
        const = ctx.enter_context(tc.tile_pool(name="const", bufs=1))
        xin = ctx.enter_context(tc.tile_pool(name="xin", bufs=6))
        xbfp = ctx.enter_context(tc.tile_pool(name="xbf", bufs=6))
        xtp = ctx.enter_context(tc.tile_pool(name="xt", bufs=2))
        psum = ctx.enter_context(tc.tile_pool(name="psum", bufs=2, space="PSUM"))
        stmp = ctx.enter_context(tc.tile_pool(name="stmp", bufs=4))

        wproj = const.tile([128, 4 * 6 * 128], bf16)
        nc.sync.dma_start(out=wproj, in_=wproj_d)
        whh = const.tile([128, 512], bf16)
        nc.sync.dma_start(out=whh, in_=whh_d)
        biasl = const.tile([1, 512], bf16)
        nc.sync.dma_start(out=biasl, in_=biasl_d)
        w1t = const.tile([128, 64], f32)
        nc.sync.dma_start(out=w1t, in_=w1t_d)
        b1 = const.tile([64, 1], f32)
        nc.sync.dma_start(out=b1, in_=b1_d)
        w2t = const.tile([64, 32], f32)
        nc.sync.dma_start(out=w2t, in_=w2t_d)
        b2 = const.tile([32, 1], f32)
        nc.sync.dma_start(out=b2, in_=b2_d)
        w3t = const.tile([32, 1], f32)
        nc.sync.dma_start(out=w3t, in_=w3t_d)
        b3 = const.tile([1, 1], f32)
        nc.sync.dma_start(out=b3, in_=b3_d)

        ident = const.tile([128, 128], bf16)
        make_identity(nc, ident)
        ones = const.tile([1, 256], bf16)
        nc.vector.memset(ones, 1.0)

        h_bf = const.tile([128, BC], bf16)
        nc.vector.memset(h_bf, 0.0)
        c_st = const.tile([128, BC], f32)
        nc.vector.memset(c_st, 0.0)
        h_f32 = const.tile([128, BC], f32)

        # prewarm the sigmoid/tanh table set so the ~2.7us load overlaps DMA
        warm = const.tile([128, 1], f32)
        nc.scalar.activation(out=warm, in_=c_st[:, 0:1], func=AF.Sigmoid)

        for gi in range(NG):
            xg = psum.tile([128, 2048], f32, tag="xg")
            stage = xg.bitcast(bf16)  # [128, 4096] bf16 view for transpose staging
            xt = xtp.tile([128, 4 * 6 * 128], bf16, tag="xt")
            for jj in range(4):
                p_, hh = jj // 2, jj % 2
                xtile = xin.tile([128, D], f32, tag="xtile")
                nc.sync.dma_start(
                    out=xtile, in_=x_r[p_, 8 * gi + 4 * hh : 8 * gi + 4 * hh + 4, :, :]
                )
                xb = xbfp.tile([128, D], bf16, tag="xb")
                nc.gpsimd.tensor_copy(out=xb, in_=xtile)
                slot = (jj % 2) * 1024
                for k in range(6):
                    nc.tensor.transpose(
                        out=stage[:, slot + k * 128 : slot + (k + 1) * 128],
                        in_=xb[:, k * 128 : (k + 1) * 128],
                        identity=ident,
                    )
                nc.vector.tensor_copy(
                    out=xt[:, jj * 768 : (jj + 1) * 768],
                    in_=stage[:, slot : slot + 768],
                )

            # projection: xg[:, 1024*p + 256*g + 32*sp + b] += W_ih^T-block @ x^T
            xt_r = xt.rearrange("p (j c) -> p j c", j=4)
            for p_ in range(2):
                for k in range(6):
                    rhs = xt_r[:, 2 * p_ : 2 * p_ + 2, k * 128 : (k + 1) * 128]
                    for g in range(4):
                        nc.tensor.matmul(
                            out=xg[:, 1024 * p_ + 256 * g : 1024 * p_ + 256 * (g + 1)],
                            lhsT=wproj[:, (g * 6 + k) * 128 : (g * 6 + k + 1) * 128],
                            rhs=rhs,
                            start=(k == 0),
                            stop=False,
                        )
                for g in range(4):
                    nc.tensor.matmul(
                        out=xg[:, 1024 * p_ + 256 * g : 1024 * p_ + 256 * (g + 1)],
                        lhsT=biasl[0:1, g * 128 : (g + 1) * 128],
                        rhs=ones[0:1, 0:256],
                        start=False,
                        stop=False,
                    )

            # scan: 16 steps, two interleaved half-batches
            xg_r = xg.rearrange("p (pp g c) -> p pp g c", pp=2, g=4)
            for s in range(16):
                p_, sp = s % 2, s // 2
                for eta in range(2):
                    col = sp * 32 + eta * HB
                    hs = h_bf[:, eta * HB : (eta + 1) * HB]
                    cs = c_st[:, eta * HB : (eta + 1) * HB]
                    for g in range(4):
                        nc.tensor.matmul(
                            out=xg_r[:, p_, g, col : col + HB],
                            lhsT=whh[:, g * 128 : (g + 1) * 128],
                            rhs=hs,
                            start=False,
                            stop=True,
                            skip_group_check=True,
                        )
                    sg = stmp.tile([128, 64], f32, tag="sg")
                    sg_r = sg.rearrange("p (g c) -> p g c", g=4)
                    nc.scalar.activation(
                        out=sg_r, in_=xg_r[:, p_, :, col : col + HB], func=AF.Sigmoid
                    )
                    u = stmp.tile([128, HB], f32, tag="u")
                    # u = (sig_g - 0.5) * sig_i ; c = 2u + sig_f*c
                    nc.vector.scalar_tensor_tensor(
                        out=u, in0=sg[:, 32:48], scalar=-0.5, in1=sg[:, 0:16],
                        op0=OP.add, op1=OP.mult,
                    )
                    v = stmp.tile([128, HB], f32, tag="v")
                    nc.vector.tensor_tensor(out=v, in0=sg[:, 16:32], in1=cs, op=OP.mult)
                    nc.vector.scalar_tensor_tensor(
                        out=cs, in0=u, scalar=2.0, in1=v, op0=OP.mult, op1=OP.add
                    )
                    th = stmp.tile([128, HB], f32, tag="th")
                    nc.scalar.activation(out=th, in_=cs, func=AF.Tanh)
                    nc.vector.tensor_tensor(out=hs, in0=sg[:, 48:64], in1=th, op=OP.mult)
                    if gi == NG - 1 and s == 15:
                        nc.vector.tensor_tensor(
                            out=h_f32[:, eta * HB : (eta + 1) * HB],
                            in0=sg[:, 48:64], in1=th, op=OP.mult,
                        )

        # MLP head (fp32): z1=relu(w1 h + b1); z2=relu(w2 z1 + b2); y=sig(w3 z2 + b3)
        mp = psum.tile([128, 2048], f32, tag="xg")
        z1s = const.tile([64, BC], f32)
        z2s = const.tile([32, BC], f32)
        y_sb = const.tile([1, BC], f32)
        nc.tensor.matmul(out=mp[0:64, 0:32], lhsT=w1t, rhs=h_f32, start=True, stop=True)
        nc.scalar.activation(out=z1s, in_=mp[0:64, 0:32], func=AF.Relu, bias=b1[:, 0:1])
        nc.tensor.matmul(out=mp[0:32, 512:544], lhsT=w2t, rhs=z1s, start=True, stop=True)
        nc.scalar.activation(out=z2s, in_=mp[0:32, 512:544], func=AF.Relu, bias=b2[:, 0:1])
        nc.tensor.matmul(out=mp[0:1, 1024:1056], lhsT=w3t, rhs=z2s, start=True, stop=True)
        nc.scalar.activation(out=y_sb, in_=mp[0:1, 1024:1056], func=AF.Sigmoid, bias=b3[:, 0:1])
        nc.sync.dma_start(out=y_d, in_=y_sb)

    nc.compile()
    return nc


def _prep_weights(W_ih, W_hh, b_ih, b_hh, w1, b1, w2, b2, w3, b3):
    import ml_dtypes

    bf16 = ml_dtypes.bfloat16
    W_ih = np.asarray(W_ih, np.float32).copy()
    W_hh = np.asarray(W_hh, np.float32).copy()
    bias = (np.asarray(b_ih, np.float32) + np.asarray(b_hh, np.float32)).copy()
    # fold the tanh-gate 2x prescale (gate order i,f,g,o -> rows 256:384)
    W_ih[256:384] *= 2.0
    W_hh[256:384] *= 2.0
    bias[256:384] *= 2.0

    wt = W_ih.T  # [768, 512]
    wproj = np.empty((128, 4 * 6 * 128), np.float32)
    for g in range(4):
        for k in range(6):
            wproj[:, (g * 6 + k) * 128 : (g * 6 + k + 1) * 128] = wt[
                k * 128 : (k + 1) * 128, g * 128 : (g + 1) * 128
            ]
    whh = W_hh.T.copy()  # [128, 512]; cols g*128+m = W_hh[128g+m, :]

    return {
        "wproj": wproj.astype(bf16),
        "whh": whh.astype(bf16),
        "biasl": bias[None, :].astype(bf16),
        "w1t": np.ascontiguousarray(np.asarray(w1, np.float32).T),
        "b1": np.asarray(b1, np.float32)[:, None].copy(),
        "w2t": np.ascontiguousarray(np.asarray(w2, np.float32).T),
        "b2": np.asarray(b2, np.float32)[:, None].copy(),
        "w3t": np.ascontiguousarray(np.asarray(w3, np.float32).T),
        "b3": np.asarray(b3, np.float32)[:, None].copy(),
    }


def _run(x, weights, trace=False, trace_kwargs=None):
    from concourse.bass_utils import run_bass_kernel_spmd

    if "nc" not in _cache:
        _cache["nc"] = _build()
    nc = _cache["nc"]

    x = np.asarray(x, np.float32)
    in_maps = []
    for kcore in range(NCORES):
        m = dict(weights)
        m["x"] = np.ascontiguousarray(x[kcore * BC : (kcore + 1) * BC])
        in_maps.append(m)
    res = run_bass_kernel_spmd(
        nc, in_maps, core_ids=list(range(NCORES)), trace=trace,
        **(trace_kwargs or {}),
    )
    out = np.empty((B, 1), np.float32)
    for kcore in range(NCORES):
        out[kcore * BC : (kcore + 1) * BC, 0] = np.asarray(
            res.results[kcore]["y"]
        ).reshape(-1)
    return out, res


def kernel(x, W_ih, W_hh, b_ih, b_hh, w1, b1, w2, b2, w3, b3):
    key = "w"
    if key not in _cache:
        _cache[key] = _prep_weights(W_ih, W_hh, b_ih, b_hh, w1, b1, w2, b2, w3, b3)
    out, _ = _run(x, _cache[key])
    return out
